# revision 1
# baseline (speedup 1.0000x reference)
"""DSRA model (chunked delta-rule linear attention + vocab projection) on 8 TRN2
NeuronCores via Bass/Tile.

Sharding (hardcoded): 8 cores = 2 batch elements x 4 vocab quarters. Core
c = 4*b + q computes batch element b's full hidden state (redundantly across
the 4 cores of that batch) and the logits for vocab columns
[q*8000, (q+1)*8000).

Device layout: "feature-major" tensors keep the model dim D=1024 on SBUF
partitions as 8 tiles of 128; tokens live on the free axis. All large GEMMs
run as float32r (FP22) matmuls, which stream at full PE rate with ~13 mantissa
bits. The causal local-context sum (4 shifted adds) is fused into the
embedding transpose as a single banded-matrix matmul. LayerNorm statistics are
partition-reductions done with ones-vector matmuls; the per-token inverse
stddev is folded into the logits PSUM->SBUF eviction as a per-partition scale.
The reference's fp32 variance overflow (h grows to ~1e20 by the last chunks,
so sum((h-mu)^2) -> inf and rsqrt -> 0) is reproduced exactly with an
is-finite mask on an unscaled fp32 variance, while the finite-path variance is
computed at a 2^-24 pre-scale for accuracy.
"""

import math
import numpy as np

import concourse.bass as bass
import concourse.mybir as mybir
import concourse.tile as tile
from concourse import bacc
from concourse.masks import make_identity

F32 = mybir.dt.float32
F32R = mybir.dt.float32r
I32 = mybir.dt.int32
AF = mybir.ActivationFunctionType
ALU = mybir.AluOpType

VOCAB, D, K, KR, CHUNK, LCTX, LAM = 32000, 1024, 128, 8, 256, 4, 0.9
S = 2048
P = 128
ND = D // P          # 8 d-tiles
NCH = S // CHUNK     # 8 chunks
NI = S // P          # 16 token blocks
VS = VOCAB // 4      # 8000 vocab per core
UC = 500             # vocab free chunk
NU = VS // UC        # 16
SCALE = 1.0 / math.sqrt(K)
EPS = 1e-5
ALPHA = 2.0 ** -24   # pre-scale for h^2 stats: late-chunk h reaches ~1e20, h^2 overflows fp32


def build_nc(debug_outputs=False, psa_bufs=4, psv_bufs=2, ctx_bufs=2, wout_bufs=3, skip_logits=False, nch=NCH, reps=1):
    nc = bacc.Bacc(None, target_bir_lowering=False, debug=False)

    xs = nc.declare_dram_parameter("xs", [S], I32, isOutput=False)
    emb = nc.declare_dram_parameter("emb", [VOCAB, D], F32, isOutput=False)
    wq = nc.declare_dram_parameter("wq", [D, K], F32, isOutput=False)
    wk = nc.declare_dram_parameter("wk", [D, K], F32, isOutput=False)
    wv = nc.declare_dram_parameter("wv", [D, D], F32, isOutput=False)
    wo = nc.declare_dram_parameter("wo", [D, D], F32, isOutput=False)
    ub = nc.declare_dram_parameter("ub", [D, KR], F32, isOutput=False)
    vb = nc.declare_dram_parameter("vb", [KR, D], F32, isOutput=False)
    lng = nc.declare_dram_parameter("lng", [D], F32, isOutput=False)
    wout = nc.declare_dram_parameter("wout", [D, VS], F32, isOutput=False)
    out = nc.declare_dram_parameter("out", [S, VS], F32, isOutput=True)

    dbg = {}
    if debug_outputs:
        dbg["ctx0"] = nc.declare_dram_parameter("dbg_ctx0", [P, ND, CHUNK], F32, isOutput=True)
        dbg["h"] = nc.declare_dram_parameter("dbg_h", [P, ND, S], F32, isOutput=True)
        dbg["r"] = nc.declare_dram_parameter("dbg_r", [S], F32, isOutput=True)

    # feature-major rearranges of the weight DRAM tensors (d = kt*128 + p)
    wq_r = wq.rearrange("(kt p) k -> p kt k", p=P)
    wk_r = wk.rearrange("(kt p) k -> p kt k", p=P)
    wv_r = wv.rearrange("(kt p) d -> p kt d", p=P)
    wo_r = wo.rearrange("(kt p) d -> p kt d", p=P)
    ub_r = ub.rearrange("(kt p) k -> p kt k", p=P)
    lng_r = lng.rearrange("(kt p) -> p kt", p=P)
    wout_r = wout.rearrange("(kt p) v -> p kt v", p=P)
    xs_r = xs.rearrange("(n p) -> p n", p=P)
    out_r = out.rearrange("(i p) v -> i p v", p=P)

    with tile.TileContext(nc) as tc:
      for _rep in range(reps):
        with (
            tc.tile_pool(name="const", bufs=1) as cpool,
            tc.tile_pool(name="persist", bufs=1) as ppool,
            tc.tile_pool(name="dramp", bufs=1, space="DRAM") as dpool,
            tc.tile_pool(name="psA", bufs=psa_bufs, space="PSUM") as psA,
            tc.tile_pool(name="psV", bufs=psv_bufs, space="PSUM") as psV,
            tc.tile_pool(name="psT", bufs=2, space="PSUM") as psT,
        ):
            # ---- constants (f32r tiles must be produced by a rounding op,
            # and Memset can't write f32r: stage in F32, then copy) ----
            ident_f = cpool.tile([P, P], F32)
            make_identity(nc, ident_f[:])
            ident = cpool.tile([P, P], F32R)
            nc.vector.tensor_copy(ident[:], ident_f[:])
            # band matrix: Bb[r, u] = 1 iff 0 <= (u - 128) - r <= LCTX-1
            bband_f = cpool.tile([P, 512], F32)
            nc.vector.memset(bband_f[:], 1.0)
            nc.gpsimd.affine_select(
                out=bband_f[:], in_=bband_f[:], pattern=[[1, 512]], base=-128,
                channel_multiplier=-1, compare_op=ALU.is_ge, fill=0.0)
            nc.gpsimd.affine_select(
                out=bband_f[:], in_=bband_f[:], pattern=[[-1, 512]], base=128 + (LCTX - 1),
                channel_multiplier=1, compare_op=ALU.is_ge, fill=0.0)
            bband = cpool.tile([P, 512], F32R)
            nc.vector.tensor_copy(bband[:], bband_f[:])
            ones_col_f = cpool.tile([P, 1], F32)
            nc.vector.memset(ones_col_f[:], 1.0 / D)
            ones_col = cpool.tile([P, 1], F32R)   # value 1/D for LN mean matmuls
            nc.vector.tensor_copy(ones_col[:], ones_col_f[:])
            one1_f = cpool.tile([P, 1], F32)
            nc.vector.memset(one1_f[:], 1.0)
            one1_col = cpool.tile([P, 1], F32R)   # value 1.0 for LN var matmuls
            nc.vector.tensor_copy(one1_col[:], one1_f[:])
            neg_row_f = cpool.tile([1, P], F32)
            nc.vector.memset(neg_row_f[:], -1.0)
            neg_row = cpool.tile([1, P], F32R)    # -1 row for -mu broadcast
            nc.vector.tensor_copy(neg_row[:], neg_row_f[:])
            lns_col = cpool.tile([P, 1], F32)     # ln(SCALE) bias for Exp
            nc.vector.memset(lns_col[:], math.log(SCALE))
            zero_col = cpool.tile([P, 1], F32)
            nc.vector.memset(zero_col[:], 0.0)
            eps1 = cpool.tile([1, 1], F32)
            nc.vector.memset(eps1[:], EPS * ALPHA * ALPHA)
            ch_scr = dpool.tile([P, ND, S], F32, name="ch_scr")
            r_scr = dpool.tile([S], F32, name="r_scr")

            # ---- small weights (persist whole kernel) ----
            xs_sb = ppool.tile([P, NI], I32)
            nc.sync.dma_start(xs_sb[:], xs_r[:, :])
            ub_sb = ppool.tile([P, ND, KR], F32)
            nc.sync.dma_start(ub_sb[:], ub_r)
            vb_sb = ppool.tile([KR, D], F32)
            nc.sync.dma_start(vb_sb[:], vb[:])
            g_cols = ppool.tile([P, ND], F32)
            nc.sync.dma_start(g_cols[:], lng_r)
            r_row = ppool.tile([1, S], F32)

            # ============================ scan phase ============================
            with (
                tc.tile_pool(name="wbig", bufs=1) as wpool,
                tc.tile_pool(name="scan", bufs=2) as spool,
                tc.tile_pool(name="etm", bufs=3) as epool,
            ):
                wq_sb = wpool.tile([P, ND, K], F32R)
                nc.sync.dma_start(wq_sb[:], wq_r.bitcast(F32R))
                wk_sb = wpool.tile([P, ND, K], F32R)
                nc.sync.dma_start(wk_sb[:], wk_r.bitcast(F32R))
                wv_t = []
                wo_t = []
                for kt in range(ND):
                    wvk = wpool.tile([P, D], F32R, name=f"wv{kt}")
                    nc.sync.dma_start(wvk[:], wv_r[:, kt, :].bitcast(F32R))
                    wv_t.append(wvk)
                for kt in range(ND):
                    wok = wpool.tile([P, D], F32R, name=f"wo{kt}")
                    nc.sync.dma_start(wok[:], wo_r[:, kt, :].bitcast(F32R))
                    wo_t.append(wok)

                # recurrent state
                S_sb = wpool.tile([P, D], F32R)
                zhalf = wpool.tile([P, 512], F32)
                nc.vector.memset(zhalf[:], 0.0)
                nc.vector.tensor_copy(S_sb[:, :512], zhalf[:])
                nc.vector.tensor_copy(S_sb[:, 512:], zhalf[:])
                St_cols = wpool.tile([P, ND], F32)
                nc.vector.memset(St_cols[:], 0.0)
                addvec = wpool.tile([P, ND], F32, name="addvec0")
                nc.vector.memset(addvec[:], 0.0)

                prev_etm1 = None
                for c in range(nch):
                    # ---- gather embeddings for this chunk (token-major) ----
                    etm0 = epool.tile([P, D], F32R, tag="etm", name=f"etm{c}_0")
                    etm1 = epool.tile([P, D], F32R, tag="etm", name=f"etm{c}_1")
                    nc.gpsimd.indirect_dma_start(
                        out=etm0[:], out_offset=None, in_=emb[:].bitcast(F32R),
                        in_offset=bass.IndirectOffsetOnAxis(ap=xs_sb[:, 2 * c:2 * c + 1], axis=0))
                    nc.gpsimd.indirect_dma_start(
                        out=etm1[:], out_offset=None, in_=emb[:].bitcast(F32R),
                        in_offset=bass.IndirectOffsetOnAxis(ap=xs_sb[:, 2 * c + 1:2 * c + 2], axis=0))

                    # ---- ctxT: transpose + causal local-context sum via band matmul ----
                    ctxt = spool.tile([P, ND, CHUNK], F32R, tag="ctx", bufs=ctx_bufs)
                    xm_cols = spool.tile([P, ND], F32, tag="xm")
                    for kt in range(ND):
                        pc = psA.tile([P, CHUNK], F32, tag="ps256", name="pc")
                        nc.tensor.matmul(pc[:], etm0[:, kt * P:(kt + 1) * P], bband[:, 128:384],
                                         start=True, stop=False)
                        nc.tensor.matmul(pc[:], etm1[:, kt * P:(kt + 1) * P], bband[:, 0:256],
                                         start=False, stop=(c == 0))
                        if c > 0:
                            nc.tensor.matmul(pc[:], prev_etm1[:, kt * P:(kt + 1) * P],
                                             bband[:, 256:512], start=False, stop=True)
                        nc.any.tensor_copy(ctxt[:, kt, :], pc[:])
                        nc.vector.tensor_reduce(out=xm_cols[:, kt:kt + 1], in_=pc[:],
                                                axis=mybir.AxisListType.X, op=ALU.add)
                    prev_etm1 = etm1
                    xmean = spool.tile([P, ND], F32, tag="xmean")
                    nc.vector.tensor_scalar_mul(xmean[:], xm_cols[:], 1.0 / CHUNK)
                    if debug_outputs and c == 0:
                        nc.sync.dma_start(dbg["ctx0"][:], ctxt[:].bitcast(F32))

                    # ---- q/k projections + phi ----
                    pq = psA.tile([P, CHUNK], F32, tag="ps256", name="pq")
                    pk = psA.tile([P, CHUNK], F32, tag="ps256", name="pk")
                    for kt in range(ND):
                        nc.tensor.matmul(pq[:], wq_sb[:, kt, :], ctxt[:, kt, :],
                                         start=(kt == 0), stop=(kt == ND - 1))
                    for kt in range(ND):
                        nc.tensor.matmul(pk[:], wk_sb[:, kt, :], ctxt[:, kt, :],
                                         start=(kt == 0), stop=(kt == ND - 1))
                    # qTs = SCALE * (elu(q)+1) = exp(min(q,0)+ln s) + s*max(q,0)
                    tmin = spool.tile([P, CHUNK], F32, tag="tmin")
                    texp = spool.tile([P, CHUNK], F32, tag="texp")
                    trel = spool.tile([P, CHUNK], F32, tag="trel")
                    qTs = spool.tile([P, CHUNK], F32R, tag="qTs")
                    nc.vector.tensor_scalar_min(tmin[:], pq[:], 0.0)
                    nc.scalar.activation(texp[:], tmin[:], AF.Exp, bias=lns_col[:])
                    nc.vector.tensor_scalar(trel[:], pq[:], 0.0, SCALE, op0=ALU.max, op1=ALU.mult)
                    nc.vector.tensor_tensor(qTs[:], texp[:], trel[:], op=ALU.add)
                    # kTp = elu(k)+1 ; kTn = -SCALE * kTp
                    tmin2 = spool.tile([P, CHUNK], F32, tag="tmin")
                    texp2 = spool.tile([P, CHUNK], F32, tag="texp")
                    trel2 = spool.tile([P, CHUNK], F32, tag="trel")
                    kTp = spool.tile([P, CHUNK], F32R, tag="kTp")
                    kTn = spool.tile([P, CHUNK], F32R, tag="kTn")
                    nc.vector.tensor_scalar_min(tmin2[:], pk[:], 0.0)
                    nc.scalar.activation(texp2[:], tmin2[:], AF.Exp, bias=zero_col[:])
                    nc.vector.tensor_scalar_max(trel2[:], pk[:], 0.0)
                    nc.vector.tensor_tensor(kTp[:], texp2[:], trel2[:], op=ALU.add)
                    nc.vector.tensor_scalar_mul(kTn[:], kTp[:], -SCALE)

                    # ---- k token-major via PE transpose ----
                    k_tm = spool.tile([P, 2, K], F32R, tag="ktm")
                    for blk in range(2):
                        pt = psA.tile([P, P], F32R, tag="ps256", name="pt")
                        nc.tensor.transpose(pt[:], kTp[:, blk * P:(blk + 1) * P], ident[:])
                        nc.any.tensor_copy(k_tm[:, blk, :], pt[:])

                    # ---- v = ctx @ Wv (token-major) and vmp = v - pred ----
                    v_sb = spool.tile([P, 2, D], F32R, tag="v")
                    vmp = spool.tile([P, 2, D], F32R, tag="vmp")
                    for i in range(2):
                        for fc in range(2):
                            pv = psV.tile([P, 512], F32, tag="ps512", name="pv")
                            for kt in range(ND):
                                nc.tensor.matmul(pv[:], ctxt[:, kt, i * P:(i + 1) * P],
                                                 wv_t[kt][:, fc * 512:(fc + 1) * 512],
                                                 start=(kt == 0), stop=False)
                            nc.any.tensor_copy(v_sb[:, i, fc * 512:(fc + 1) * 512], pv[:])
                            nc.tensor.matmul(pv[:], kTn[:, i * P:(i + 1) * P],
                                             S_sb[:, fc * 512:(fc + 1) * 512],
                                             start=False, stop=True)
                            nc.any.tensor_copy(vmp[:, i, fc * 512:(fc + 1) * 512], pv[:])

                    # ---- attnT[j, i] = sum_K kTp[K,j] * qTs[K,i], mask j<=i ----
                    attnT = spool.tile([P, 2, CHUNK], F32R, tag="attn")
                    for j in range(2):
                        pa = psA.tile([P, CHUNK], F32, tag="ps256", name="pa")
                        nc.tensor.matmul(pa[:], kTp[:, j * P:(j + 1) * P], qTs[:],
                                         start=True, stop=True)
                        nc.vector.tensor_copy(attnT[:, j, :], pa[:])
                        nc.gpsimd.affine_select(
                            out=attnT[:, j, :], in_=attnT[:, j, :], pattern=[[1, CHUNK]],
                            base=-(j * P), channel_multiplier=-1, compare_op=ALU.is_ge, fill=0.0)

                    # ---- out_pre (feature-major) = v^T@attnT + S^T@qTs + addvec ----
                    opre = spool.tile([P, ND, CHUNK], F32R, tag="opre", bufs=1)
                    for kt in range(ND):
                        po = psA.tile([P, CHUNK], F32, tag="ps256", name="po")
                        nc.tensor.matmul(po[:], v_sb[:, 0, kt * P:(kt + 1) * P], attnT[:, 0, :],
                                         start=True, stop=False)
                        nc.tensor.matmul(po[:], v_sb[:, 1, kt * P:(kt + 1) * P], attnT[:, 1, :],
                                         start=False, stop=False)
                        nc.tensor.matmul(po[:], S_sb[:, kt * P:(kt + 1) * P], qTs[:],
                                         start=False, stop=True)
                        nc.vector.tensor_scalar(opre[:, kt, :], po[:], addvec[:, kt:kt + 1], None,
                                                op0=ALU.add)

                    # ---- h chunk = Wo^T @ out_pre (feature-major), LN stats, spill ----
                    hch = spool.tile([P, ND, CHUNK], F32R, tag="hch", bufs=1)
                    for d2 in range(ND):
                        ph = psA.tile([P, CHUNK], F32, tag="ps256", name="ph")
                        for kt in range(ND):
                            nc.tensor.matmul(ph[:], wo_t[kt][:, d2 * P:(d2 + 1) * P],
                                             opre[:, kt, :], start=(kt == 0), stop=(kt == ND - 1))
                        nc.any.tensor_copy(hch[:, d2, :], ph[:])
                    if debug_outputs:
                        nc.sync.dma_start(dbg["h"][:, :, c * CHUNK:(c + 1) * CHUNK],
                                          hch[:].bitcast(F32))

                    # mean over D via ones-matmul (partition reduction)
                    pmu = psT.tile([1, CHUNK], F32, tag="pstiny", name="pmu")
                    for kt in range(ND):
                        nc.tensor.matmul(pmu[:], ones_col[:], hch[:, kt, :],
                                         start=(kt == 0), stop=(kt == ND - 1))
                    mu_row = spool.tile([1, CHUNK], F32R, tag="mur", bufs=1)
                    nc.vector.tensor_copy(mu_row[:], pmu[:])
                    # -mu broadcast over partitions, then ch = h - mu (spill to DRAM)
                    pb = psA.tile([P, CHUNK], F32, tag="ps256", name="pb")
                    nc.tensor.matmul(pb[:], neg_row[:], mu_row[:], start=True, stop=True)
                    chs = spool.tile([P, ND, CHUNK], F32R, tag="chs", bufs=1)
                    for kt in range(ND):
                        nc.vector.tensor_tensor(chs[:, kt, :], hch[:, kt, :].bitcast(F32), pb[:],
                                                op=ALU.add)
                    nc.sync.dma_start(ch_scr[:, :, c * CHUNK:(c + 1) * CHUNK], chs[:].bitcast(F32))

                    # var = mean(ch^2), twice: unscaled fp32 (reproduces the reference's
                    # overflow-to-inf -> rsqrt = 0) and ALPHA-prescaled (accurate value).
                    psq = psT.tile([1, CHUNK], F32, tag="pstiny", name="psq")
                    psqs = psT.tile([1, CHUNK], F32, tag="pstiny", name="psqs")
                    for kt in range(ND):
                        csq = spool.tile([P, CHUNK], F32R, tag="hsq")
                        nc.scalar.activation(csq[:], chs[:, kt, :].bitcast(F32), AF.Square,
                                             bias=zero_col[:])
                        nc.tensor.matmul(psq[:], one1_col[:], csq[:],
                                         start=(kt == 0), stop=(kt == ND - 1))
                    for kt in range(ND):
                        csqs = spool.tile([P, CHUNK], F32R, tag="hsq")
                        nc.scalar.activation(csqs[:], chs[:, kt, :].bitcast(F32), AF.Square,
                                             bias=zero_col[:], scale=ALPHA)
                        nc.tensor.matmul(psqs[:], one1_col[:], csqs[:],
                                         start=(kt == 0), stop=(kt == ND - 1))
                    mask_row = spool.tile([1, CHUNK], F32, tag="maskr", bufs=1)
                    nc.vector.tensor_scalar(mask_row[:], psq[:], 3.4028234663852886e38, None, op0=ALU.is_le)
                    var_row = spool.tile([1, CHUNK], F32, tag="varr", bufs=1)
                    nc.vector.tensor_scalar_mul(var_row[:], psqs[:], 1.0 / D)
                    sd_row = spool.tile([1, CHUNK], F32, tag="sdr", bufs=1)
                    nc.scalar.activation(sd_row[:], var_row[:], AF.Sqrt, bias=eps1[:])
                    tmp_r = spool.tile([1, CHUNK], F32, tag="tmpr", bufs=1)
                    nc.vector.reciprocal(tmp_r[:], sd_row[:])
                    nc.vector.tensor_scalar_mul(tmp_r[:], tmp_r[:], ALPHA)
                    nc.vector.tensor_tensor(r_row[:, c * CHUNK:(c + 1) * CHUNK], tmp_r[:],
                                            mask_row[:], op=ALU.mult)

                    # ---- S update: S += k_tm^T @ vmp ----
                    for fc in range(2):
                        pS = psV.tile([P, 512], F32, tag="ps512", name="pS")
                        nc.tensor.matmul(pS[:], k_tm[:, 0, :], vmp[:, 0, fc * 512:(fc + 1) * 512],
                                         start=True, stop=False)
                        nc.tensor.matmul(pS[:], k_tm[:, 1, :], vmp[:, 1, fc * 512:(fc + 1) * 512],
                                         start=False, stop=True)
                        nc.vector.tensor_tensor(S_sb[:, fc * 512:(fc + 1) * 512],
                                                S_sb[:, fc * 512:(fc + 1) * 512].bitcast(F32),
                                                pS[:], op=ALU.add)

                    # ---- bypass + time state for next chunk ----
                    pbt = psT.tile([KR, 1], F32, tag="pstiny", name="pbt")
                    for kt in range(ND):
                        nc.tensor.matmul(pbt[:], ub_sb[:, kt, :], xmean[:, kt:kt + 1],
                                         start=(kt == 0), stop=(kt == ND - 1))
                    bypT = spool.tile([KR, 1], F32, tag="bypT")
                    nc.vector.tensor_copy(bypT[:], pbt[:])
                    pbv = psT.tile([P, ND], F32, tag="pstiny", name="pbv")
                    for kt in range(ND):
                        nc.tensor.matmul(pbv[:, kt:kt + 1], vb_sb[:, kt * P:(kt + 1) * P],
                                         bypT[:], start=True, stop=True)
                    t1 = spool.tile([P, ND], F32, tag="t1")
                    nc.vector.tensor_scalar_mul(t1[:], xmean[:], 1.0 - LAM)
                    nc.vector.tensor_scalar_mul(St_cols[:], St_cols[:], LAM)
                    nc.vector.tensor_tensor(St_cols[:], St_cols[:], t1[:], op=ALU.add)
                    addvec = wpool.tile([P, ND], F32, name=f"addvec{c + 1}", tag="addv", bufs=2)
                    nc.vector.tensor_tensor(addvec[:], St_cols[:], pbv[:], op=ALU.add)

            # r_row -> token-major r_col via DRAM bounce
            nc.sync.dma_start(r_scr[:][None, :], r_row[:])
            if debug_outputs:
                nc.sync.dma_start(dbg["r"][None, :], r_row[:])

            # ============================ logits phase ============================
            if skip_logits:
                lg_range = []
            else:
                lg_range = range(NU)
            with (
                tc.tile_pool(name="chp", bufs=1) as chpool,
                tc.tile_pool(name="wop", bufs=3) as wopool,
                tc.tile_pool(name="osb", bufs=4) as opool,
            ):
                chsb = chpool.tile([P, ND, S], F32R)
                nc.sync.dma_start(chsb[:], ch_scr[:].bitcast(F32R))
                r_col = chpool.tile([P, NI], F32)
                nc.sync.dma_start(r_col[:], r_scr[:].rearrange("(i p) -> p i", p=P))
                # fold ln_g (per-feature) into ch
                for kt in range(ND):
                    nc.vector.tensor_scalar_mul(chsb[:, kt, :], chsb[:, kt, :].bitcast(F32),
                                                g_cols[:, kt:kt + 1])
                for u in lg_range:
                    wsb = wopool.tile([P, ND, UC], F32R, tag="wout", bufs=wout_bufs)
                    nc.sync.dma_start(wsb[:], wout_r[:, :, u * UC:(u + 1) * UC].bitcast(F32R))
                    for i in range(NI):
                        pm = psA.tile([P, UC], F32, tag="ps256", name="pm")
                        for kt in range(ND):
                            nc.tensor.matmul(pm[:], chsb[:, kt, i * P:(i + 1) * P],
                                             wsb[:, kt, :], start=(kt == 0), stop=(kt == ND - 1))
                        osb = opool.tile([P, UC], F32, tag="osb")
                        if i % 2 == 0:
                            nc.vector.tensor_scalar_mul(osb[:], pm[:], r_col[:, i:i + 1])
                        else:
                            nc.scalar.activation(osb[:], pm[:], AF.Copy, scale=r_col[:, i:i + 1])
                        nc.sync.dma_start(out_r[i, :, u * UC:(u + 1) * UC], osb[:])

    nc.compile()
    return nc


def make_in_maps(inputs):
    """Full inputs dict -> list of 8 per-core input maps."""
    x = np.asarray(inputs["x"])
    f = lambda k: np.ascontiguousarray(np.asarray(inputs[k], dtype=np.float32))
    emb, Wq, Wk, Wv, Wo = f("emb_table"), f("Wq"), f("Wk"), f("Wv"), f("Wo")
    Ub, Vb, ln_g, Wout = f("Ub"), f("Vb"), f("ln_g"), f("Wout")
    in_maps = []
    for c in range(8):
        b, q = c // 4, c % 4
        in_maps.append({
            "xs": np.ascontiguousarray(x[b].astype(np.int32)),
            "emb": emb, "wq": Wq, "wk": Wk, "wv": Wv, "wo": Wo,
            "ub": Ub, "vb": Vb, "lng": ln_g,
            "wout": np.ascontiguousarray(Wout[:, q * VS:(q + 1) * VS]),
        })
    return in_maps


def assemble(results):
    out = np.empty((2, S, VOCAB), np.float32)
    for c in range(8):
        b, q = c // 4, c % 4
        out[b, :, q * VS:(q + 1) * VS] = results[c]["out"]
    return out


_NC_CACHE = None


def kernel(**inputs) -> np.ndarray:
    """Full (unsharded) inputs -> full [2, 2048, 32000] float32 logits."""
    global _NC_CACHE
    from concourse.bass_utils import run_bass_kernel_spmd
    if _NC_CACHE is None:
        _NC_CACHE = build_nc()
    in_maps = make_in_maps(inputs)
    res = run_bass_kernel_spmd(_NC_CACHE, in_maps, core_ids=list(range(8)))
    return assemble(res.results)

